# revision 4
# baseline (speedup 1.0000x reference)
import os
import sys

sys.path.insert(0, "/opt/trn_rl_repo")

import numpy as np

import concourse.bass as bass
import concourse.tile as tile
import concourse.mybir as mybir
from concourse import bacc
from concourse.bass import ts
from concourse.bass_utils import run_bass_kernel_spmd

N_CORES = 8
C = 32
SIZE = 128
N_FULL = 50000

SCALE_P = 63.5                 # (size-1)/2
DELTA_P = 0.0625 * 63.5        # sample spacing in pixel units = 3.96875

F32 = mybir.dt.float32
F16 = mybir.dt.float16
I32 = mybir.dt.int32

AluOp = mybir.AluOpType
ActFn = mybir.ActivationFunctionType

# x-pair offsets within the gathered 10-voxel span per x-class
CLASS_OFFS = [(0, 4, 8), (0, 3, 7), (0, 4, 7), (0, 3, 6)]
CLASS_R = [(4, 8), (3, 7), (4, 7), (3, 6)]

NQ = 2               # SWDGE queues for indirect gathers
TRACE = False
LAST_RESULT = None

_cache = {}


def _emit_tile(nc, tc, pools, t, tl_out_row, offs, consts):
    """Emit one 128-vertex tile. t: tile index into the batched prologue
    tensors; offs: x-pair offsets in the 10-voxel span for this class."""
    (gpool, spool, fpool, ftpool, pspool, opool) = pools
    (mb_sb, w9, idxi, s6, vol, out) = consts

    def wcol(kidx, axis):
        col = t * 9 + kidx * 3 + axis
        return w9[:, col : col + 1]

    # 9 runs of [10 x][4 basis][32 c] = 1280 f16 els per (kz, ky)
    G = gpool.tile([128, 9, 1280], F16, tag="G")
    for j in range(9):
        bi = nc.gpsimd.indirect_dma_start(
            out=G[:, j, :],
            out_offset=None,
            in_=vol[:, :],
            in_offset=bass.IndirectOffsetOnAxis(
                ap=idxi[:, t * 9 + j : t * 9 + j + 1], axis=0
            ),
        )
        if NQ > 1 and j % NQ:
            bi.ins.queue = f"qPoolDynamic{j % NQ}"

    Gv = G[:].rearrange("p j (x b c) -> p j x b c", x=10, c=C)
    # F10[j, x, c] = a + wy*dy + wz*(dz + wy*dzy)   (bilinear basis eval)
    U = spool.tile([128, 9 * 320], F16, tag="U")     # dzy*wz
    T = spool.tile([128, 9 * 320], F16, tag="T")     # dz*wz
    for kz in range(3):
        nc.vector.tensor_scalar(
            U[:, kz * 960 : (kz + 1) * 960],
            Gv[:, kz * 3 : (kz + 1) * 3, :, 3, :],
            wcol(kz, 2), None, AluOp.mult,
        )
        nc.vector.tensor_scalar(
            T[:, kz * 960 : (kz + 1) * 960],
            Gv[:, kz * 3 : (kz + 1) * 3, :, 2, :],
            wcol(kz, 2), None, AluOp.mult,
        )
    S = spool.tile([128, 9 * 320], F16, tag="S")
    nc.vector.tensor_tensor(
        S[:], Gv[:, :, :, 1, :], U[:].rearrange("p (a b) -> p a b", b=320),
        AluOp.add,
    )
    Sv = S[:].rearrange("p (kz ky r) -> p kz ky r", kz=3, ky=3)
    for ky in range(3):
        nc.scalar.activation(
            Sv[:, :, ky], Sv[:, :, ky], ActFn.Copy, bias=0.0,
            scale=wcol(ky, 1),
        )
    Q = spool.tile([128, 9 * 320], F16, tag="Q")
    nc.vector.tensor_tensor(
        Q[:], Gv[:, :, :, 0, :], T[:].rearrange("p (a b) -> p a b", b=320),
        AluOp.add,
    )
    F10 = spool.tile([128, 9 * 320], F16, tag="F10")
    nc.vector.tensor_tensor(F10[:], Q[:], S[:], AluOp.add)

    # x-extract + x-lerp fold: F[j, kx, xl, c] = s6[kx, xl] * F10[j, offs[kx]+xl, c]
    F = fpool.tile([128, 14 * 128], F16, tag="F")
    F10v = F10[:].rearrange("p (j x c) -> p j x c", x=10, c=C)
    Fv = F[:, 0 : 27 * 64].rearrange("p (j kx xl c) -> p j kx xl c", kx=3, xl=2, c=C)
    for kx in range(3):
        for xl in range(2):
            col = t * 6 + kx * 2 + xl
            sc = s6[:, col : col + 1]
            src = F10v[:, :, offs[kx] + xl, :]
            dst = Fv[:, :, kx, xl, :]
            if kx == 1:
                nc.scalar.activation(dst, src, ActFn.Copy, bias=0.0, scale=sc)
            else:
                nc.vector.tensor_scalar(dst, src, sc, None, AluOp.mult)
    nc.vector.memset(F[:, 1728:1729], 1.0)
    nc.vector.memset(F[:, 1729:1792], 0.0)

    FT = ftpool.tile([128, 14, 128], F16, tag="FT")
    nc.sync.dma_start_transpose(FT[:], F[:])

    psum = pspool.tile([128, C], F32, tag="ps")
    for m in range(14):
        nc.tensor.matmul(
            psum[:], FT[:, m, :], mb_sb[:, ts(m, C)],
            start=(m == 0), stop=(m == 13),
        )
    osb = opool.tile([128, C], F32, tag="osb")
    nc.scalar.activation(osb[:], psum[:], ActFn.Copy, bias=0.0)
    nc.sync.dma_start(out[ts(tl_out_row, 128), :], osb[:])


def _build(tile_counts):
    """tile_counts: per-class 128-vertex tile counts."""
    tiles = sum(tile_counts)
    nv = tiles * 128
    nc = bacc.Bacc(
        "TRN2", target_bir_lowering=False, debug=False, num_swdge_queues=NQ
    )

    vol = nc.dram_tensor(
        "vol", [SIZE * SIZE * SIZE, 128], F16, kind="ExternalInput"
    ).ap()  # rows (z*128+y)*128+x of [basis(4), c(32)]
    verts = nc.dram_tensor("verts", [nv, 3], F32, kind="ExternalInput").ap()
    mbig = nc.dram_tensor("mbig", [128, 14 * C], F16, kind="ExternalInput").ap()
    out = nc.dram_tensor("out", [nv, C], F32, kind="ExternalOutput").ap()

    with tile.TileContext(nc) as tc:
        with (
            tc.tile_pool(name="const", bufs=1) as cpool,
            tc.tile_pool(name="gather", bufs=3) as gpool,
            tc.tile_pool(name="scr", bufs=2) as spool,
            tc.tile_pool(name="fl", bufs=3) as fpool,
            tc.tile_pool(name="ft", bufs=3) as ftpool,
            tc.tile_pool(name="psum", bufs=4, space="PSUM") as pspool,
            tc.tile_pool(name="outp", bufs=3) as opool,
        ):
            mb_sb = cpool.tile([128, 14 * C], F16, tag="mb")
            nc.sync.dma_start(mb_sb[:], mbig[:])
            vall = cpool.tile([128, tiles * 3], F32, tag="vall")
            nc.sync.dma_start(vall[:], verts.rearrange("(t p) a -> p t a", p=128))

            # ---- batched prologue ----
            p9 = cpool.tile([128, tiles * 9], F32, tag="p9")
            p9v = p9[:].rearrange("p (t k a) -> p t k a", k=3, a=3)
            vv = vall[:].rearrange("p (t a) -> p t a", a=3)
            for k in range(3):
                nc.scalar.activation(
                    p9v[:, :, k, :], vv, ActFn.Copy,
                    bias=SCALE_P + (k - 1) * DELTA_P, scale=SCALE_P,
                )
            ci = cpool.tile([128, tiles * 9], I32, tag="ci")
            nc.vector.tensor_copy(ci[:], p9[:])
            cf = cpool.tile([128, tiles * 9], F32, tag="cf")
            nc.vector.tensor_copy(cf[:], ci[:])
            d9 = cpool.tile([128, tiles * 9], F32, tag="d9")
            nc.vector.tensor_tensor(d9[:], p9[:], cf[:], AluOp.subtract)
            m9 = cpool.tile([128, tiles * 9], F32, tag="m9")
            nc.vector.tensor_scalar(m9[:], d9[:], 0.0, None, AluOp.is_lt)
            w9 = cpool.tile([128, tiles * 9], F32, tag="w9")
            nc.vector.tensor_tensor(w9[:], d9[:], m9[:], AluOp.add)
            i9 = cpool.tile([128, tiles * 9], F32, tag="i9")
            nc.vector.tensor_tensor(i9[:], cf[:], m9[:], AluOp.subtract)

            i9v = i9[:].rearrange("p (t k a) -> p t k a", k=3, a=3)
            w9v = w9[:].rearrange("p (t k a) -> p t k a", k=3, a=3)

            # run base index: idx[t, kz, ky] = 16384*z0(kz) + 128*y0(ky) + x0(0)
            zs = cpool.tile([128, tiles * 3], F32, tag="zs")
            zsv = zs[:].rearrange("p (t z) -> p t z", z=3)
            nc.vector.tensor_scalar(
                zsv, i9v[:, :, :, 2], 16384.0, None, AluOp.mult
            )
            zy = cpool.tile([128, tiles * 9], F32, tag="zy")
            zyv = zy[:].rearrange("p (t z y) -> p t z y", z=3, y=3)
            y0 = i9v[:, :, :, 1]
            for kz in range(3):
                zsb = zsv[:, :, kz].unsqueeze(2).broadcast_to([128, tiles, 3])
                nc.vector.scalar_tensor_tensor(
                    zyv[:, :, kz, :], y0, 128.0, zsb, AluOp.mult, AluOp.add
                )
            idxf = cpool.tile([128, tiles * 9], F32, tag="idxf")
            x0b = (
                i9v[:, :, 0, 0].unsqueeze(2).broadcast_to([128, tiles, 9])
            )
            nc.vector.tensor_tensor(
                idxf[:].rearrange("p (t z) -> p t z", z=9),
                zy[:].rearrange("p (t z) -> p t z", z=9),
                x0b, AluOp.add,
            )
            idxi = cpool.tile([128, tiles * 9], I32, tag="idxi")
            nc.vector.tensor_copy(idxi[:], idxf[:])

            # x-fold scales: s6[t, kx, xl] = xl ? wx : 1-wx
            s6 = cpool.tile([128, tiles * 6], F32, tag="s6")
            s6v = s6[:].rearrange("p (t x l) -> p t x l", x=3, l=2)
            wx = w9v[:, :, :, 0]
            nc.vector.tensor_copy(s6v[:, :, :, 1], wx)
            nc.vector.tensor_scalar(
                s6v[:, :, :, 0], wx, -1.0, 1.0, AluOp.mult, AluOp.add
            )

            pools = (gpool, spool, fpool, ftpool, pspool, opool)
            consts = (mb_sb, w9, idxi, s6, vol, out)
            tl = 0
            for cls, n_t in enumerate(tile_counts):
                for _ in range(n_t):
                    _emit_tile(nc, tc, pools, tl, tl, CLASS_OFFS[cls], consts)
                    tl += 1

    nc.compile()
    return nc


def _get_nc(tile_counts):
    key = tuple(tile_counts)
    if key not in _cache:
        _cache[key] = _build(key)
    return _cache[key]


def _host_prep(voxel_features, vertices, w_d1, b_d1, w_d2, b_d2,
               w_c1, b_c1, w_c2, b_c2, conv_w, conv_b):
    # volume rows [z, y, x] of [basis(4), c(32)] f16; basis = bilinear corner
    # basis in (z, y): (a, dy, dz, dzy)
    v = np.transpose(np.asarray(voxel_features, np.float32)[0], (1, 2, 3, 0))
    v = np.ascontiguousarray(v)  # [z, y, x, c] f32
    vp = np.empty((SIZE + 1, SIZE + 1, SIZE, C), np.float32)
    vp[:SIZE, :SIZE] = v
    vp[SIZE, :SIZE] = v[SIZE - 1]
    vp[:, SIZE] = vp[:, SIZE - 1]
    a = vp[:SIZE, :SIZE]
    dy = vp[:SIZE, 1:] - a
    dz = vp[1:, :SIZE] - a
    dzy = vp[1:, 1:] - vp[1:, :SIZE] - vp[:SIZE, 1:] + a
    vol4b = np.empty((SIZE, SIZE, SIZE, 4, C), np.float16)
    vol4b[:, :, :, 0] = a
    vol4b[:, :, :, 1] = dy
    vol4b[:, :, :, 2] = dz
    vol4b[:, :, :, 3] = dzy
    vol4b = vol4b.reshape(SIZE * SIZE * SIZE, 128)

    f8 = np.float64
    Wd = np.asarray(w_d2, f8) @ np.asarray(w_d1, f8)
    bd = np.asarray(b_d1, f8) @ np.asarray(w_d2, f8).T + np.asarray(b_d2, f8)
    Wc = np.asarray(w_c2, f8) @ np.asarray(w_c1, f8)
    bc = np.asarray(b_c1, f8) @ np.asarray(w_c2, f8).T + np.asarray(b_c2, f8)
    cw = np.asarray(conv_w, f8)[:, :, 0, :]  # [o, c', k]

    A = np.einsum("ock,cd->odk", cw, Wd)  # [o, c, k]
    M = np.moveaxis(A, 2, 0).copy()  # [k, o, c], ref order k = kx*9 + ky*3 + kz
    M[13] += Wc - A.sum(axis=2)
    bias_tot = cw.sum(axis=2) @ bd + np.asarray(conv_b, f8) + bc

    # f-dim layout: ((kz*3+ky)*6 + kx*2 + xl)*32 + c ; row 1728 = bias (F=1)
    Mbig = np.zeros((14 * 128, C), np.float64)
    for kz in range(3):
        for ky in range(3):
            for kx in range(3):
                base = ((kz * 3 + ky) * 3 + kx) * 64
                k = kx * 9 + ky * 3 + kz
                Mbig[base : base + 32] = M[k].T
                Mbig[base + 32 : base + 64] = M[k].T
    Mbig[1728] = bias_tot
    mb_host = np.ascontiguousarray(
        Mbig.reshape(14, 128, C).transpose(1, 0, 2).reshape(128, 14 * C)
    ).astype(np.float16)
    return vol4b, mb_host


def _classify(vp):
    """vp: [n, 3] f32 vertices -> x-class id, replicating the device's f32
    arithmetic exactly."""
    q = vp[:, 0].astype(np.float32) * np.float32(SCALE_P)
    x0 = np.floor(q + np.float32(SCALE_P - DELTA_P)).astype(np.int64)
    x1 = np.floor(q + np.float32(SCALE_P)).astype(np.int64)
    x2 = np.floor(q + np.float32(SCALE_P + DELTA_P)).astype(np.int64)
    r1 = x1 - x0
    r2 = x2 - x0
    cls = np.full(vp.shape[0], -1, np.int64)
    for i, (ra, rb) in enumerate(CLASS_R):
        cls[(r1 == ra) & (r2 == rb)] = i
    assert (cls >= 0).all(), "unexpected x-spacing class"
    return cls


def kernel(**inputs):
    global LAST_RESULT
    vol4b, mb_host = _host_prep(**inputs)
    vp = np.asarray(inputs["vertices"], np.float32)[0]
    n = vp.shape[0]
    per = (n + N_CORES - 1) // N_CORES
    dev_cores = int(os.environ.get("K_DEV_CORES", "0")) or N_CORES

    # shard contiguously, then class-sort within each core
    in_maps_meta = []
    counts_ref = None
    for i in range(dev_cores):
        seg = vp[i * per : min((i + 1) * per, n)]
        cls = _classify(seg)
        order = np.argsort(cls, kind="stable")
        seg_sorted = seg[order]
        cls_sorted = cls[order]
        tile_counts = []
        v_parts = []
        for c in range(len(CLASS_OFFS)):
            part = seg_sorted[cls_sorted == c]
            n_t = (len(part) + 127) // 128
            tile_counts.append(n_t)
            v_parts.append(part)
        if counts_ref is None:
            counts_ref = tuple(tile_counts)
        else:
            counts_ref = tuple(max(a, b) for a, b in zip(counts_ref, tile_counts))
        in_maps_meta.append({
            "parts": v_parts, "order": order, "seg_len": len(seg),
            "cls_counts": [len(p) for p in v_parts],
        })

    # pad every core's class segments to the shared per-class tile counts
    in_maps = []
    for m in in_maps_meta:
        pieces = []
        for c in range(len(CLASS_OFFS)):
            part = m["parts"][c]
            need = counts_ref[c] * 128
            if len(part) < need:
                fill = part[:1] if len(part) else vp[:1]
                part = np.concatenate(
                    [part, np.repeat(fill, need - len(part), axis=0)], axis=0
                )
            pieces.append(part)
        verts_padded = np.ascontiguousarray(
            np.concatenate(pieces, axis=0), np.float32
        )
        in_maps.append({"vol": vol4b, "verts": verts_padded, "mbig": mb_host})

    nc = _get_nc(counts_ref)
    kwargs = {}
    if TRACE:
        kwargs = {"trace": True, "trace_cores": [0]}
    res = run_bass_kernel_spmd(nc, in_maps, list(range(dev_cores)), **kwargs)
    LAST_RESULT = res

    out = np.zeros((n, C), np.float32)
    bounds = np.cumsum([0] + [c * 128 for c in counts_ref])
    for i in range(dev_cores):
        m = in_maps_meta[i]
        raw = res.results[i]["out"]
        vals = []
        for c in range(len(CLASS_OFFS)):
            k = m["cls_counts"][c]
            vals.append(raw[bounds[c] : bounds[c] + k])
        sorted_out = np.concatenate(vals, axis=0)
        seg_out = np.empty_like(sorted_out)
        seg_out[m["order"]] = sorted_out
        lo = i * per
        out[lo : lo + m["seg_len"]] = seg_out
    return out.reshape(1, n, C)
